# revision 10
# baseline (speedup 1.0000x reference)
import sys
sys.path.insert(0, '/opt/trn_rl_repo')
import numpy as np
import concourse.bass as bass
import concourse.bacc as bacc
import concourse.tile as tile
import concourse.mybir as mybir
from concourse.bass_utils import run_bass_kernel_spmd

C3_TABLE = [(0, 1, 2), (1, 2, 3), (2, 3, 4), (3, 4, 5), (0, 4, 5), (0, 1, 5),
            (0, 1, 2, 3), (1, 2, 3, 4), (2, 3, 4, 5), (0, 3, 4, 5), (0, 1, 4, 5),
            (0, 1, 2, 5), (0, 1, 3, 4), (1, 2, 4, 5), (0, 2, 3, 5),
            (0, 1, 2, 3, 4, 5)]
A = 1.7159
S = 2.0 / 3.0

B, C, H, W = 256, 6, 142, 142
KH = KW = 5
OC = 16
OH, OW = H - 4, W - 4          # 138
NCORES = 8
B_LOC = B // NCORES            # 32
T = 6                          # oh rows per block
HH = T + KH - 1                # 10
NS2 = 2                        # kw parity packed into K (s dim)
RA = C * HH                    # 60 s0 rows
K = 64 + RA                    # 124: 0..59 s0, 60..63 zero, 64..123 s1 (x shifted +1 col)
M = OC * T                     # 96
NP = 3                         # passes: kw pairs {0,1},{2,3},{4,-}
BPER = 2
NS = BPER * OW                 # 276
NBLK = OH // T                 # 23 exactly
NPAIR = B_LOC // BPER          # 16
HP = 4                         # pairs per psum group
NQ = NPAIR // HP               # 4 psum groups per block
XF = B_LOC * W                 # 4544
SFS = NPAIR * NS               # 4416

_cache = {}


def _build():
    if 'nc' in _cache:
        return _cache['nc']
    f32 = mybir.dt.float32
    f16 = mybir.dt.float16
    Tanh = mybir.ActivationFunctionType.Tanh
    nc = bacc.Bacc("TRN2", target_bir_lowering=False, debug=False,
                   num_devices=NCORES)
    x_d = nc.dram_tensor("x", [NBLK, 64, B_LOC, W], f16, kind="ExternalInput").ap()
    w_d = nc.dram_tensor("w", [K, NP * M], f16, kind="ExternalInput").ap()
    b_d = nc.dram_tensor("b", [M, 1], f32, kind="ExternalInput").ap()
    y_d = nc.dram_tensor("y", [NBLK, M, SFS], f16, kind="ExternalOutput").ap()

    with tile.TileContext(nc) as tc:
        with tc.tile_pool(name="wpool", bufs=1) as wpool, \
             tc.tile_pool(name="xpool", bufs=4) as xpool, \
             tc.tile_pool(name="spool", bufs=4) as spool, \
             tc.tile_pool(name="pspool", bufs=1, space="PSUM") as pspool:
            # weights on the scalar HWDGE queue so the first x block loads in
            # parallel on the sync queue
            w_sb = wpool.tile([K, NP * M], f16)
            nc.scalar.dma_start(w_sb[:], w_d[:])
            b_sb = wpool.tile([M, 1], f32)
            nc.scalar.dma_start(b_sb[:], b_d[:])

            for blk in range(NBLK):
                xt = xpool.tile([K, XF], f16)
                xq = nc.sync if blk % 2 == 0 else nc.gpsimd
                xq.dma_start(xt[0:64],
                             x_d[blk].rearrange("k i w -> k (i w)"))
                # build s1 rows: x shifted one col (col w holds x[w+1])
                nc.vector.tensor_copy(xt[64:64 + RA, 0:XF - 1], xt[0:RA, 1:XF])
                nc.vector.tensor_copy(xt[64:64 + RA, XF - 1:XF],
                                      xt[0:RA, XF - 1:XF])
                stage = spool.tile([M, NQ, HP, NS], f16)
                xv = xt[:].rearrange("k (i w) -> k i w", i=B_LOC)
                for q in range(NQ):
                    ps = pspool.tile([M, HP, 512], f32, name=f"ps{q % 2}",
                                     tag=f"ps{q % 2}")
                    for g in range(NP):
                        for p_ in range(HP):
                            pair = q * HP + p_
                            rv = xv[:, pair * BPER:(pair + 1) * BPER, :]
                            nc.tensor.matmul(
                                ps[:, p_, 0:NS],
                                w_sb[:, g * M:(g + 1) * M],
                                rv[:, :, 2 * g:2 * g + OW],
                                start=(g == 0), stop=(g == NP - 1),
                            )
                    nc.scalar.activation(stage[:, q], ps[:, :, 0:NS],
                                         Tanh, bias=b_sb[:], scale=S)
                    if q % 2 == 1:
                        # flush each completed half of the stage immediately
                        h = q // 2
                        nc.scalar.dma_start(
                            y_d[blk, :, h * (SFS // 2):(h + 1) * (SFS // 2)],
                            stage[:, 2 * h:2 * h + 2]
                            .rearrange("m q p n -> m (q p n)"))
    nc.compile()
    _cache['nc'] = nc
    return nc


def _prep_weights(w3, b3, w4, b4, w6, b6):
    Wd = np.zeros((OC, C, KH, KW), np.float32)
    bias = np.zeros((OC,), np.float32)
    for i, idx in enumerate(C3_TABLE[:6]):
        Wd[i, list(idx)] = w3[i]
        bias[i] = b3[i]
    for i, idx in enumerate(C3_TABLE[6:15]):
        Wd[6 + i, list(idx)] = w4[i]
        bias[6 + i] = b4[i]
    Wd[15, list(C3_TABLE[15])] = w6[0]
    bias[15] = b6[0]

    # K row r = s*64 + c*HH + hh ; col m = oc*T + j ; pass g: kw = 2g+s
    wk = np.zeros((K, NP, M), np.float32)
    for c in range(C):
        for hh in range(HH):
            for j in range(T):
                kh = hh - j
                if not (0 <= kh < KH):
                    continue
                for s in range(NS2):
                    for g in range(NP):
                        kw = 2 * g + s
                        if kw < KW:
                            r = s * 64 + c * HH + hh
                            wk[r, g, np.arange(OC) * T + j] = Wd[:, c, kh, kw]
    bvec = (S * bias[np.arange(M) // T]).reshape(M, 1).astype(np.float32)
    return wk.reshape(K, NP * M).astype(np.float16), bvec


def _prep_x(x_shard):
    # [B_LOC, C, H, W] -> [NBLK, 64, B_LOC, W]: s0 rows (c,hh), 4 zero pad rows
    xt = np.ascontiguousarray(x_shard.transpose(1, 2, 0, 3)).astype(np.float16)
    xb = np.zeros((NBLK, 64, B_LOC, W), np.float16)
    for blk in range(NBLK):
        r0 = blk * T
        xb[blk, 0:RA] = xt[:, r0:r0 + HH].reshape(RA, B_LOC, W)
    return xb


def prep_in_maps(x, w3, b3, w4, b4, w6, b6):
    w3, b3, w4, b4, w6, b6 = [np.asarray(a, dtype=np.float32)
                              for a in (w3, b3, w4, b4, w6, b6)]
    wk, bvec = _prep_weights(w3, b3, w4, b4, w6, b6)
    x = np.ascontiguousarray(np.asarray(x), dtype=np.float32)
    return [{"x": _prep_x(x[i * B_LOC:(i + 1) * B_LOC]), "w": wk, "b": bvec}
            for i in range(NCORES)]


def _unpack_y(y_s):
    # y_s [NBLK, M, SFS]; SFS = (pair16, e2, ow); m = oc*T + j
    v = y_s.reshape(NBLK, OC, T, NPAIR, BPER, OW).astype(np.float32)
    v = v.transpose(3, 4, 1, 0, 2, 5)               # pair,e,oc,blk,j,ow
    return v.reshape(B_LOC, OC, OH, OW)


def kernel(x, w3, b3, w4, b4, w6, b6):
    nc = _build()
    in_maps = prep_in_maps(x, w3, b3, w4, b4, w6, b6)
    res = run_bass_kernel_spmd(nc, in_maps, list(range(NCORES)))
    out = np.concatenate([A * _unpack_y(res.results[i]["y"])
                          for i in range(NCORES)], axis=0)
    return np.ascontiguousarray(out)


# revision 12
# speedup vs baseline: 1.0470x; 1.0470x over previous
import sys
sys.path.insert(0, '/opt/trn_rl_repo')
import numpy as np
import concourse.bass as bass
import concourse.bacc as bacc
import concourse.tile as tile
import concourse.mybir as mybir
from concourse.bass_utils import run_bass_kernel_spmd

C3_TABLE = [(0, 1, 2), (1, 2, 3), (2, 3, 4), (3, 4, 5), (0, 4, 5), (0, 1, 5),
            (0, 1, 2, 3), (1, 2, 3, 4), (2, 3, 4, 5), (0, 3, 4, 5), (0, 1, 4, 5),
            (0, 1, 2, 5), (0, 1, 3, 4), (1, 2, 4, 5), (0, 2, 3, 5),
            (0, 1, 2, 3, 4, 5)]
A = 1.7159
S = 2.0 / 3.0

B, C, H, W = 256, 6, 142, 142
KH = KW = 5
OC = 16
OH, OW = H - 4, W - 4          # 138
NCORES = 8
B_LOC = B // NCORES            # 32
T = 6                          # oh rows per block
HH = T + KH - 1                # 10
NS2 = 2                        # kw parity packed into K (s dim)
RA = C * HH                    # 60 s0 rows
K = 64 + RA                    # 124: 0..59 s0, 60..63 zero, 64..123 s1 (x shifted +1 col)
M = OC * T                     # 96
NP = 3                         # passes: kw pairs {0,1},{2,3},{4,-}
BPER = 2
NS = BPER * OW                 # 276
NBLK = OH // T                 # 23 exactly
NPAIR = B_LOC // BPER          # 16
HP = 4                         # pairs per psum group
NQ = NPAIR // HP               # 4 psum groups per block
XF = B_LOC * W                 # 4544
SFS = NPAIR * NS               # 4416

_cache = {}


def _build():
    if 'nc' in _cache:
        return _cache['nc']
    f32 = mybir.dt.float32
    f16 = mybir.dt.float16
    Tanh = mybir.ActivationFunctionType.Tanh
    nc = bacc.Bacc("TRN2", target_bir_lowering=False, debug=False,
                   num_devices=NCORES)
    x_d = nc.dram_tensor("x", [NBLK, 64, B_LOC, W], f16, kind="ExternalInput").ap()
    w_d = nc.dram_tensor("w", [K, NP * M], f16, kind="ExternalInput").ap()
    b_d = nc.dram_tensor("b", [M, 1], f32, kind="ExternalInput").ap()
    y_d = nc.dram_tensor("y", [NBLK, M, SFS], f16, kind="ExternalOutput").ap()

    with tile.TileContext(nc) as tc:
        with tc.tile_pool(name="wpool", bufs=1) as wpool, \
             tc.tile_pool(name="xpool", bufs=4) as xpool, \
             tc.tile_pool(name="spool", bufs=4) as spool, \
             tc.tile_pool(name="pspool", bufs=1, space="PSUM") as pspool:
            # weights on the scalar HWDGE queue so the first x block loads in
            # parallel on the sync queue
            w_sb = wpool.tile([K, NP * M], f16)
            nc.scalar.dma_start(w_sb[:], w_d[:])
            b_sb = wpool.tile([M, 1], f32)
            nc.scalar.dma_start(b_sb[:], b_d[:])

            for blk in range(NBLK):
                xt = xpool.tile([K, XF], f16)
                nc.sync.dma_start(xt[0:64],
                                  x_d[blk].rearrange("k i w -> k (i w)"))
                # build s1 rows: x shifted one col (col w holds x[w+1])
                nc.vector.tensor_copy(xt[64:64 + RA, 0:XF - 1], xt[0:RA, 1:XF])
                nc.vector.tensor_copy(xt[64:64 + RA, XF - 1:XF],
                                      xt[0:RA, XF - 1:XF])
                stage = spool.tile([M, NQ, HP, NS], f16)
                xv = xt[:].rearrange("k (i w) -> k i w", i=B_LOC)
                for q in range(NQ):
                    ps = pspool.tile([M, HP, 512], f32, name=f"ps{q % 2}",
                                     tag=f"ps{q % 2}")
                    for g in range(NP):
                        for p_ in range(HP):
                            pair = q * HP + p_
                            rv = xv[:, pair * BPER:(pair + 1) * BPER, :]
                            nc.tensor.matmul(
                                ps[:, p_, 0:NS],
                                w_sb[:, g * M:(g + 1) * M],
                                rv[:, :, 2 * g:2 * g + OW],
                                start=(g == 0), stop=(g == NP - 1),
                            )
                    nc.scalar.activation(stage[:, q], ps[:, :, 0:NS],
                                         Tanh, bias=b_sb[:], scale=S)
                # y on the gpsimd SWDGE queue: scalar engine runs activations
                # only, so PSUM recycling never waits on a DMA issue
                nc.gpsimd.dma_start(y_d[blk],
                                    stage[:].rearrange("m q p n -> m (q p n)"))
    nc.compile()
    _cache['nc'] = nc
    return nc


def _prep_weights(w3, b3, w4, b4, w6, b6):
    Wd = np.zeros((OC, C, KH, KW), np.float32)
    bias = np.zeros((OC,), np.float32)
    for i, idx in enumerate(C3_TABLE[:6]):
        Wd[i, list(idx)] = w3[i]
        bias[i] = b3[i]
    for i, idx in enumerate(C3_TABLE[6:15]):
        Wd[6 + i, list(idx)] = w4[i]
        bias[6 + i] = b4[i]
    Wd[15, list(C3_TABLE[15])] = w6[0]
    bias[15] = b6[0]

    # K row r = s*64 + c*HH + hh ; col m = oc*T + j ; pass g: kw = 2g+s
    wk = np.zeros((K, NP, M), np.float32)
    for c in range(C):
        for hh in range(HH):
            for j in range(T):
                kh = hh - j
                if not (0 <= kh < KH):
                    continue
                for s in range(NS2):
                    for g in range(NP):
                        kw = 2 * g + s
                        if kw < KW:
                            r = s * 64 + c * HH + hh
                            wk[r, g, np.arange(OC) * T + j] = Wd[:, c, kh, kw]
    bvec = (S * bias[np.arange(M) // T]).reshape(M, 1).astype(np.float32)
    return wk.reshape(K, NP * M).astype(np.float16), bvec


def _prep_x(x_shard):
    # [B_LOC, C, H, W] -> [NBLK, 64, B_LOC, W]: s0 rows (c,hh), 4 zero pad rows
    xt = np.ascontiguousarray(x_shard.transpose(1, 2, 0, 3)).astype(np.float16)
    xb = np.zeros((NBLK, 64, B_LOC, W), np.float16)
    for blk in range(NBLK):
        r0 = blk * T
        xb[blk, 0:RA] = xt[:, r0:r0 + HH].reshape(RA, B_LOC, W)
    return xb


def prep_in_maps(x, w3, b3, w4, b4, w6, b6):
    w3, b3, w4, b4, w6, b6 = [np.asarray(a, dtype=np.float32)
                              for a in (w3, b3, w4, b4, w6, b6)]
    wk, bvec = _prep_weights(w3, b3, w4, b4, w6, b6)
    x = np.ascontiguousarray(np.asarray(x), dtype=np.float32)
    return [{"x": _prep_x(x[i * B_LOC:(i + 1) * B_LOC]), "w": wk, "b": bvec}
            for i in range(NCORES)]


def _unpack_y(y_s):
    # y_s [NBLK, M, SFS]; SFS = (pair16, e2, ow); m = oc*T + j
    v = y_s.reshape(NBLK, OC, T, NPAIR, BPER, OW).astype(np.float32)
    v = v.transpose(3, 4, 1, 0, 2, 5)               # pair,e,oc,blk,j,ow
    return v.reshape(B_LOC, OC, OH, OW)


def kernel(x, w3, b3, w4, b4, w6, b6):
    nc = _build()
    in_maps = prep_in_maps(x, w3, b3, w4, b4, w6, b6)
    res = run_bass_kernel_spmd(nc, in_maps, list(range(NCORES)))
    out = np.concatenate([A * _unpack_y(res.results[i]["y"])
                          for i in range(NCORES)], axis=0)
    return np.ascontiguousarray(out)


# revision 14
# speedup vs baseline: 1.0612x; 1.0136x over previous
import sys
sys.path.insert(0, '/opt/trn_rl_repo')
import numpy as np
import concourse.bass as bass
import concourse.bacc as bacc
import concourse.tile as tile
import concourse.mybir as mybir
from concourse.bass_utils import run_bass_kernel_spmd

C3_TABLE = [(0, 1, 2), (1, 2, 3), (2, 3, 4), (3, 4, 5), (0, 4, 5), (0, 1, 5),
            (0, 1, 2, 3), (1, 2, 3, 4), (2, 3, 4, 5), (0, 3, 4, 5), (0, 1, 4, 5),
            (0, 1, 2, 5), (0, 1, 3, 4), (1, 2, 4, 5), (0, 2, 3, 5),
            (0, 1, 2, 3, 4, 5)]
A = 1.7159
S = 2.0 / 3.0

B, C, H, W = 256, 6, 142, 142
KH = KW = 5
OC = 16
OH, OW = H - 4, W - 4          # 138
NCORES = 8
B_LOC = B // NCORES            # 32
T = 6                          # oh rows per block
HH = T + KH - 1                # 10
NS2 = 2                        # kw parity packed into K (s dim)
RA = C * HH                    # 60 s0 rows
K = 64 + RA                    # 124: 0..59 s0, 60..63 zero, 64..123 s1 (x shifted +1 col)
M = OC * T                     # 96
NP = 3                         # passes: kw pairs {0,1},{2,3},{4,-}
BPER = 2
NS = BPER * OW                 # 276
NBLK = OH // T                 # 23 exactly
NPAIR = B_LOC // BPER          # 16
HP = 4                         # pairs per psum group
NQ = NPAIR // HP               # 4 psum groups per block
XF = B_LOC * W                 # 4544
SFS = NPAIR * NS               # 4416

_cache = {}


def _build():
    if 'nc' in _cache:
        return _cache['nc']
    f32 = mybir.dt.float32
    f16 = mybir.dt.float16
    Tanh = mybir.ActivationFunctionType.Tanh
    nc = bacc.Bacc("TRN2", target_bir_lowering=False, debug=False,
                   num_devices=NCORES)
    x_d = nc.dram_tensor("x", [NBLK, 64, B_LOC, W], f16, kind="ExternalInput").ap()
    w_d = nc.dram_tensor("w", [K, NP * M], f16, kind="ExternalInput").ap()
    b_d = nc.dram_tensor("b", [M, 1], f32, kind="ExternalInput").ap()
    y_d = nc.dram_tensor("y", [NBLK, M, SFS], f16, kind="ExternalOutput").ap()

    with tile.TileContext(nc) as tc:
        with tc.tile_pool(name="wpool", bufs=1) as wpool, \
             tc.tile_pool(name="xpool", bufs=4) as xpool, \
             tc.tile_pool(name="spool", bufs=4) as spool, \
             tc.tile_pool(name="pspool", bufs=1, space="PSUM") as pspool:
            # weights on the scalar HWDGE queue so the first x block loads in
            # parallel on the sync queue
            w_sb = wpool.tile([K, NP * M], f16)
            nc.scalar.dma_start(w_sb[:], w_d[:])
            b_sb = wpool.tile([M, 1], f32)
            nc.scalar.dma_start(b_sb[:], b_d[:])

            HXF = XF // 2

            def load_shift(xt, blk, split):
                # DMA s0 rows then build s1 rows (x shifted one col left).
                # Shift runs within batch-halves; the two boundary columns
                # (w=141 of i=15 and i=31) get unshifted values, which only
                # ever multiply the zero kw=5 weights.
                src = x_d[blk].rearrange("k i w -> k (i w)")
                if split:
                    nc.sync.dma_start(xt[0:64, 0:HXF], src[:, 0:HXF])
                    nc.gpsimd.dma_start(xt[0:64, HXF:XF], src[:, HXF:XF])
                    bounds = [(0, HXF), (HXF, XF)]
                else:
                    nc.sync.dma_start(xt[0:64], src)
                    bounds = [(0, XF)]
                for lo, hi in bounds:
                    nc.vector.tensor_copy(xt[64:64 + RA, lo:hi - 1],
                                          xt[0:RA, lo + 1:hi])
                    nc.vector.tensor_copy(xt[64:64 + RA, hi - 1:hi],
                                          xt[0:RA, hi - 1:hi])

            for blk in range(NBLK):
                xt = xpool.tile([K, XF], f16)
                load_shift(xt, blk, split=(blk < 3))
                stage = spool.tile([M, NQ, HP, NS], f16)
                xv = xt[:].rearrange("k (i w) -> k i w", i=B_LOC)
                for q in range(NQ):
                    ps = pspool.tile([M, HP, 512], f32, name=f"ps{q % 2}",
                                     tag=f"ps{q % 2}")
                    for g in range(NP):
                        for p_ in range(HP):
                            pair = q * HP + p_
                            rv = xv[:, pair * BPER:(pair + 1) * BPER, :]
                            nc.tensor.matmul(
                                ps[:, p_, 0:NS],
                                w_sb[:, g * M:(g + 1) * M],
                                rv[:, :, 2 * g:2 * g + OW],
                                start=(g == 0), stop=(g == NP - 1),
                            )
                    nc.scalar.activation(stage[:, q], ps[:, :, 0:NS],
                                         Tanh, bias=b_sb[:], scale=S)
                    if q % 2 == 1:
                        # flush each finished stage half; gpsimd SWDGE queue so
                        # the scalar engine runs activations only
                        h = q // 2
                        nc.gpsimd.dma_start(
                            y_d[blk, :, h * (SFS // 2):(h + 1) * (SFS // 2)],
                            stage[:, 2 * h:2 * h + 2]
                            .rearrange("m q p n -> m (q p n)"))
    nc.compile()
    _cache['nc'] = nc
    return nc


def _prep_weights(w3, b3, w4, b4, w6, b6):
    Wd = np.zeros((OC, C, KH, KW), np.float32)
    bias = np.zeros((OC,), np.float32)
    for i, idx in enumerate(C3_TABLE[:6]):
        Wd[i, list(idx)] = w3[i]
        bias[i] = b3[i]
    for i, idx in enumerate(C3_TABLE[6:15]):
        Wd[6 + i, list(idx)] = w4[i]
        bias[6 + i] = b4[i]
    Wd[15, list(C3_TABLE[15])] = w6[0]
    bias[15] = b6[0]

    # K row r = s*64 + c*HH + hh ; col m = oc*T + j ; pass g: kw = 2g+s
    wk = np.zeros((K, NP, M), np.float32)
    for c in range(C):
        for hh in range(HH):
            for j in range(T):
                kh = hh - j
                if not (0 <= kh < KH):
                    continue
                for s in range(NS2):
                    for g in range(NP):
                        kw = 2 * g + s
                        if kw < KW:
                            r = s * 64 + c * HH + hh
                            wk[r, g, np.arange(OC) * T + j] = Wd[:, c, kh, kw]
    bvec = (S * bias[np.arange(M) // T]).reshape(M, 1).astype(np.float32)
    return wk.reshape(K, NP * M).astype(np.float16), bvec


def _prep_x(x_shard):
    # [B_LOC, C, H, W] -> [NBLK, 64, B_LOC, W]: s0 rows (c,hh), 4 zero pad rows
    xt = np.ascontiguousarray(x_shard.transpose(1, 2, 0, 3)).astype(np.float16)
    xb = np.zeros((NBLK, 64, B_LOC, W), np.float16)
    for blk in range(NBLK):
        r0 = blk * T
        xb[blk, 0:RA] = xt[:, r0:r0 + HH].reshape(RA, B_LOC, W)
    return xb


def prep_in_maps(x, w3, b3, w4, b4, w6, b6):
    w3, b3, w4, b4, w6, b6 = [np.asarray(a, dtype=np.float32)
                              for a in (w3, b3, w4, b4, w6, b6)]
    wk, bvec = _prep_weights(w3, b3, w4, b4, w6, b6)
    x = np.ascontiguousarray(np.asarray(x), dtype=np.float32)
    return [{"x": _prep_x(x[i * B_LOC:(i + 1) * B_LOC]), "w": wk, "b": bvec}
            for i in range(NCORES)]


def _unpack_y(y_s):
    # y_s [NBLK, M, SFS]; SFS = (pair16, e2, ow); m = oc*T + j
    v = y_s.reshape(NBLK, OC, T, NPAIR, BPER, OW).astype(np.float32)
    v = v.transpose(3, 4, 1, 0, 2, 5)               # pair,e,oc,blk,j,ow
    return v.reshape(B_LOC, OC, OH, OW)


def kernel(x, w3, b3, w4, b4, w6, b6):
    nc = _build()
    in_maps = prep_in_maps(x, w3, b3, w4, b4, w6, b6)
    res = run_bass_kernel_spmd(nc, in_maps, list(range(NCORES)))
    out = np.concatenate([A * _unpack_y(res.results[i]["y"])
                          for i in range(NCORES)], axis=0)
    return np.ascontiguousarray(out)


# revision 15
# speedup vs baseline: 1.0642x; 1.0028x over previous
import sys
sys.path.insert(0, '/opt/trn_rl_repo')
import numpy as np
import concourse.bass as bass
import concourse.bacc as bacc
import concourse.tile as tile
import concourse.mybir as mybir
from concourse.bass_utils import run_bass_kernel_spmd

C3_TABLE = [(0, 1, 2), (1, 2, 3), (2, 3, 4), (3, 4, 5), (0, 4, 5), (0, 1, 5),
            (0, 1, 2, 3), (1, 2, 3, 4), (2, 3, 4, 5), (0, 3, 4, 5), (0, 1, 4, 5),
            (0, 1, 2, 5), (0, 1, 3, 4), (1, 2, 4, 5), (0, 2, 3, 5),
            (0, 1, 2, 3, 4, 5)]
A = 1.7159
S = 2.0 / 3.0

B, C, H, W = 256, 6, 142, 142
KH = KW = 5
OC = 16
OH, OW = H - 4, W - 4          # 138
NCORES = 8
B_LOC = B // NCORES            # 32
T = 6                          # oh rows per block
HH = T + KH - 1                # 10
NS2 = 2                        # kw parity packed into K (s dim)
RA = C * HH                    # 60 s0 rows
K = 64 + RA                    # 124: 0..59 s0, 60..63 zero, 64..123 s1 (x shifted +1 col)
M = OC * T                     # 96
NP = 3                         # passes: kw pairs {0,1},{2,3},{4,-}
BPER = 2
NS = BPER * OW                 # 276
NBLK = OH // T                 # 23 exactly
NPAIR = B_LOC // BPER          # 16
HP = 4                         # pairs per psum group
NQ = NPAIR // HP               # 4 psum groups per block
XF = B_LOC * W                 # 4544
SFS = NPAIR * NS               # 4416

_cache = {}


def _build():
    if 'nc' in _cache:
        return _cache['nc']
    f32 = mybir.dt.float32
    f16 = mybir.dt.float16
    Tanh = mybir.ActivationFunctionType.Tanh
    nc = bacc.Bacc("TRN2", target_bir_lowering=False, debug=False,
                   num_devices=NCORES)
    x_d = nc.dram_tensor("x", [NBLK, 64, B_LOC, W], f16, kind="ExternalInput").ap()
    w_d = nc.dram_tensor("w", [K, NP * M], f16, kind="ExternalInput").ap()
    b_d = nc.dram_tensor("b", [M, 1], f32, kind="ExternalInput").ap()
    y_d = nc.dram_tensor("y", [NBLK, M, SFS], f16, kind="ExternalOutput").ap()

    with tile.TileContext(nc) as tc:
        with tc.tile_pool(name="wpool", bufs=1) as wpool, \
             tc.tile_pool(name="xpool", bufs=4) as xpool, \
             tc.tile_pool(name="spool", bufs=4) as spool, \
             tc.tile_pool(name="pspool", bufs=1, space="PSUM") as pspool:
            # weights on the scalar HWDGE queue so the first x block loads in
            # parallel on the sync queue
            w_sb = wpool.tile([K, NP * M], f16)
            nc.scalar.dma_start(w_sb[:], w_d[:])
            b_sb = wpool.tile([M, 1], f32)
            nc.scalar.dma_start(b_sb[:], b_d[:])

            HXF = XF // 2

            def load_shift(xt, blk, split):
                # DMA s0 rows then build s1 rows (x shifted one col left).
                # Shift runs within batch-halves; the two boundary columns
                # (w=141 of i=15 and i=31) get unshifted values, which only
                # ever multiply the zero kw=5 weights.
                src = x_d[blk].rearrange("k i w -> k (i w)")
                if split:
                    nc.sync.dma_start(xt[0:64, 0:HXF], src[:, 0:HXF])
                    nc.gpsimd.dma_start(xt[0:64, HXF:XF], src[:, HXF:XF])
                    bounds = [(0, HXF), (HXF, XF)]
                else:
                    nc.sync.dma_start(xt[0:64], src)
                    bounds = [(0, XF)]
                for lo, hi in bounds:
                    nc.vector.tensor_copy(xt[64:64 + RA, lo:hi - 1],
                                          xt[0:RA, lo + 1:hi])
                    nc.vector.tensor_copy(xt[64:64 + RA, hi - 1:hi],
                                          xt[0:RA, hi - 1:hi])

            for blk in range(NBLK):
                xt = xpool.tile([K, XF], f16)
                load_shift(xt, blk, split=(blk < 3))
                stage = spool.tile([M, NQ, HP, NS], f16)
                xv = xt[:].rearrange("k (i w) -> k i w", i=B_LOC)
                for q in range(NQ):
                    ps = pspool.tile([M, HP, 512], f32, name=f"ps{q % 2}",
                                     tag=f"ps{q % 2}")
                    for g in range(NP):
                        for p_ in range(HP):
                            pair = q * HP + p_
                            rv = xv[:, pair * BPER:(pair + 1) * BPER, :]
                            nc.tensor.matmul(
                                ps[:, p_, 0:NS],
                                w_sb[:, g * M:(g + 1) * M],
                                rv[:, :, 2 * g:2 * g + OW],
                                start=(g == 0), stop=(g == NP - 1),
                            )
                    nc.scalar.activation(stage[:, q], ps[:, :, 0:NS],
                                         Tanh, bias=b_sb[:], scale=S)
                    if q % 2 == 1:
                        # flush each finished stage half off the scalar engine:
                        # first half via gpsimd SWDGE, second via sync HWDGE
                        # (hardware issue, keeps the drain tail off slow SWDGE)
                        h = q // 2
                        yq = nc.gpsimd if h == 0 else nc.sync
                        yq.dma_start(
                            y_d[blk, :, h * (SFS // 2):(h + 1) * (SFS // 2)],
                            stage[:, 2 * h:2 * h + 2]
                            .rearrange("m q p n -> m (q p n)"))
    nc.compile()
    _cache['nc'] = nc
    return nc


def _prep_weights(w3, b3, w4, b4, w6, b6):
    Wd = np.zeros((OC, C, KH, KW), np.float32)
    bias = np.zeros((OC,), np.float32)
    for i, idx in enumerate(C3_TABLE[:6]):
        Wd[i, list(idx)] = w3[i]
        bias[i] = b3[i]
    for i, idx in enumerate(C3_TABLE[6:15]):
        Wd[6 + i, list(idx)] = w4[i]
        bias[6 + i] = b4[i]
    Wd[15, list(C3_TABLE[15])] = w6[0]
    bias[15] = b6[0]

    # K row r = s*64 + c*HH + hh ; col m = oc*T + j ; pass g: kw = 2g+s
    wk = np.zeros((K, NP, M), np.float32)
    for c in range(C):
        for hh in range(HH):
            for j in range(T):
                kh = hh - j
                if not (0 <= kh < KH):
                    continue
                for s in range(NS2):
                    for g in range(NP):
                        kw = 2 * g + s
                        if kw < KW:
                            r = s * 64 + c * HH + hh
                            wk[r, g, np.arange(OC) * T + j] = Wd[:, c, kh, kw]
    bvec = (S * bias[np.arange(M) // T]).reshape(M, 1).astype(np.float32)
    return wk.reshape(K, NP * M).astype(np.float16), bvec


def _prep_x(x_shard):
    # [B_LOC, C, H, W] -> [NBLK, 64, B_LOC, W]: s0 rows (c,hh), 4 zero pad rows
    xt = np.ascontiguousarray(x_shard.transpose(1, 2, 0, 3)).astype(np.float16)
    xb = np.zeros((NBLK, 64, B_LOC, W), np.float16)
    for blk in range(NBLK):
        r0 = blk * T
        xb[blk, 0:RA] = xt[:, r0:r0 + HH].reshape(RA, B_LOC, W)
    return xb


def prep_in_maps(x, w3, b3, w4, b4, w6, b6):
    w3, b3, w4, b4, w6, b6 = [np.asarray(a, dtype=np.float32)
                              for a in (w3, b3, w4, b4, w6, b6)]
    wk, bvec = _prep_weights(w3, b3, w4, b4, w6, b6)
    x = np.ascontiguousarray(np.asarray(x), dtype=np.float32)
    return [{"x": _prep_x(x[i * B_LOC:(i + 1) * B_LOC]), "w": wk, "b": bvec}
            for i in range(NCORES)]


def _unpack_y(y_s):
    # y_s [NBLK, M, SFS]; SFS = (pair16, e2, ow); m = oc*T + j
    v = y_s.reshape(NBLK, OC, T, NPAIR, BPER, OW).astype(np.float32)
    v = v.transpose(3, 4, 1, 0, 2, 5)               # pair,e,oc,blk,j,ow
    return v.reshape(B_LOC, OC, OH, OW)


def kernel(x, w3, b3, w4, b4, w6, b6):
    nc = _build()
    in_maps = prep_in_maps(x, w3, b3, w4, b4, w6, b6)
    res = run_bass_kernel_spmd(nc, in_maps, list(range(NCORES)))
    out = np.concatenate([A * _unpack_y(res.results[i]["y"])
                          for i in range(NCORES)], axis=0)
    return np.ascontiguousarray(out)


# revision 18
# speedup vs baseline: 1.1296x; 1.0614x over previous
import sys
sys.path.insert(0, '/opt/trn_rl_repo')
import numpy as np
import concourse.bass as bass
import concourse.bacc as bacc
import concourse.tile as tile
import concourse.mybir as mybir
from concourse.bass_utils import run_bass_kernel_spmd

C3_TABLE = [(0, 1, 2), (1, 2, 3), (2, 3, 4), (3, 4, 5), (0, 4, 5), (0, 1, 5),
            (0, 1, 2, 3), (1, 2, 3, 4), (2, 3, 4, 5), (0, 3, 4, 5), (0, 1, 4, 5),
            (0, 1, 2, 5), (0, 1, 3, 4), (1, 2, 4, 5), (0, 2, 3, 5),
            (0, 1, 2, 3, 4, 5)]
A = 1.7159
S = 2.0 / 3.0

B, C, H, W = 256, 6, 142, 142
KH = KW = 5
OC = 16
OH, OW = H - 4, W - 4          # 138
NCORES = 8
B_LOC = B // NCORES            # 32
T = 6                          # oh rows per block
HH = T + KH - 1                # 10
NS2 = 2                        # kw parity packed into K (s dim)
RA = C * HH                    # 60 s0 rows
K = 64 + RA                    # 124: 0..59 s0, 60..63 zero, 64..123 s1 (x shifted +1 col)
M = OC * T                     # 96
NP = 3                         # passes: kw pairs {0,1},{2,3},{4,-}
BPER = 2
NS = BPER * OW                 # 276
NBLK = OH // T                 # 23 exactly
NPAIR = B_LOC // BPER          # 16
HP = 4                         # pairs per psum group
NQ = NPAIR // HP               # 4 psum groups per block
XF = B_LOC * W                 # 4544
SFS = NPAIR * NS               # 4416

_cache = {}


def _build():
    if 'nc' in _cache:
        return _cache['nc']
    f32 = mybir.dt.float32
    f16 = mybir.dt.float16
    Tanh = mybir.ActivationFunctionType.Tanh
    nc = bacc.Bacc("TRN2", target_bir_lowering=False, debug=False,
                   num_devices=NCORES)
    x_d = nc.dram_tensor("x", [NBLK, 64, B_LOC, W], f16, kind="ExternalInput").ap()
    w_d = nc.dram_tensor("w", [K, NP * M], f16, kind="ExternalInput").ap()
    b_d = nc.dram_tensor("b", [M, 1], f32, kind="ExternalInput").ap()
    y_d = nc.dram_tensor("y", [NBLK, M, SFS], f16, kind="ExternalOutput").ap()

    with tile.TileContext(nc) as tc:
        with tc.tile_pool(name="wpool", bufs=1) as wpool, \
             tc.tile_pool(name="xpool", bufs=4) as xpool, \
             tc.tile_pool(name="spool", bufs=4) as spool, \
             tc.tile_pool(name="pspool", bufs=1, space="PSUM") as pspool:
            # weights on the scalar HWDGE queue so the first x block loads in
            # parallel on the sync queue
            w_sb = wpool.tile([K, NP * M], f16)
            nc.scalar.dma_start(w_sb[:], w_d[:])
            b_sb = wpool.tile([M, 1], f32)
            nc.scalar.dma_start(b_sb[:], b_d[:])

            QXF = XF // 4

            def load_shift(xt, blk, split):
                # DMA s0 rows then build s1 rows (x shifted one col left).
                # Shift runs within batch-groups; each group's last column
                # (w=141 of its last batch) gets an unshifted value, which
                # only ever multiplies the zero kw=5 weights.
                src = x_d[blk].rearrange("k i w -> k (i w)")
                if split:
                    # q-group-aligned quarters over all three DMA queues so
                    # the first matmuls start after one quarter + its copy
                    qs = [nc.sync, nc.gpsimd, nc.scalar, nc.sync]
                    bounds = [(i * QXF, (i + 1) * QXF) for i in range(4)]
                    for (lo, hi), eng in zip(bounds, qs):
                        eng.dma_start(xt[0:64, lo:hi], src[:, lo:hi])
                else:
                    nc.sync.dma_start(xt[0:64], src)
                    bounds = [(0, XF)]
                for lo, hi in bounds:
                    nc.vector.tensor_copy(xt[64:64 + RA, lo:hi - 1],
                                          xt[0:RA, lo + 1:hi])
                    nc.vector.tensor_copy(xt[64:64 + RA, hi - 1:hi],
                                          xt[0:RA, hi - 1:hi])

            for blk in range(NBLK):
                xt = xpool.tile([K, XF], f16)
                load_shift(xt, blk, split=(blk < 3))
                stage = spool.tile([M, NQ, HP, NS], f16)
                xv = xt[:].rearrange("k (i w) -> k i w", i=B_LOC)
                for q in range(NQ):
                    ps = pspool.tile([M, HP, 512], f32, name=f"ps{q % 2}",
                                     tag=f"ps{q % 2}")
                    for g in range(NP):
                        for p_ in range(HP):
                            pair = q * HP + p_
                            rv = xv[:, pair * BPER:(pair + 1) * BPER, :]
                            nc.tensor.matmul(
                                ps[:, p_, 0:NS],
                                w_sb[:, g * M:(g + 1) * M],
                                rv[:, :, 2 * g:2 * g + OW],
                                start=(g == 0), stop=(g == NP - 1),
                            )
                    nc.scalar.activation(stage[:, q], ps[:, :, 0:NS],
                                         Tanh, bias=b_sb[:], scale=S)
                    if blk == NBLK - 1:
                        # last block: flush per quarter so the final transfer
                        # on the drain tail is small
                        yq = nc.gpsimd if q == 0 else nc.sync
                        yq.dma_start(
                            y_d[blk, :, q * (SFS // 4):(q + 1) * (SFS // 4)],
                            stage[:, q:q + 1].rearrange("m q p n -> m (q p n)"))
                    elif q % 2 == 1:
                        # flush each finished stage half off the scalar engine:
                        # first half via gpsimd SWDGE, second via sync HWDGE
                        # (hardware issue, keeps the drain tail off slow SWDGE)
                        h = q // 2
                        yq = nc.gpsimd if h == 0 else nc.sync
                        yq.dma_start(
                            y_d[blk, :, h * (SFS // 2):(h + 1) * (SFS // 2)],
                            stage[:, 2 * h:2 * h + 2]
                            .rearrange("m q p n -> m (q p n)"))
    nc.compile()
    _cache['nc'] = nc
    return nc


def _prep_weights(w3, b3, w4, b4, w6, b6):
    Wd = np.zeros((OC, C, KH, KW), np.float32)
    bias = np.zeros((OC,), np.float32)
    for i, idx in enumerate(C3_TABLE[:6]):
        Wd[i, list(idx)] = w3[i]
        bias[i] = b3[i]
    for i, idx in enumerate(C3_TABLE[6:15]):
        Wd[6 + i, list(idx)] = w4[i]
        bias[6 + i] = b4[i]
    Wd[15, list(C3_TABLE[15])] = w6[0]
    bias[15] = b6[0]

    # K row r = s*64 + c*HH + hh ; col m = oc*T + j ; pass g: kw = 2g+s
    wk = np.zeros((K, NP, M), np.float32)
    for c in range(C):
        for hh in range(HH):
            for j in range(T):
                kh = hh - j
                if not (0 <= kh < KH):
                    continue
                for s in range(NS2):
                    for g in range(NP):
                        kw = 2 * g + s
                        if kw < KW:
                            r = s * 64 + c * HH + hh
                            wk[r, g, np.arange(OC) * T + j] = Wd[:, c, kh, kw]
    bvec = (S * bias[np.arange(M) // T]).reshape(M, 1).astype(np.float32)
    return wk.reshape(K, NP * M).astype(np.float16), bvec


def _prep_x(x_shard):
    # [B_LOC, C, H, W] -> [NBLK, 64, B_LOC, W]: s0 rows (c,hh), 4 zero pad rows
    xt = np.ascontiguousarray(x_shard.transpose(1, 2, 0, 3)).astype(np.float16)
    xb = np.zeros((NBLK, 64, B_LOC, W), np.float16)
    for blk in range(NBLK):
        r0 = blk * T
        xb[blk, 0:RA] = xt[:, r0:r0 + HH].reshape(RA, B_LOC, W)
    return xb


def prep_in_maps(x, w3, b3, w4, b4, w6, b6):
    w3, b3, w4, b4, w6, b6 = [np.asarray(a, dtype=np.float32)
                              for a in (w3, b3, w4, b4, w6, b6)]
    wk, bvec = _prep_weights(w3, b3, w4, b4, w6, b6)
    x = np.ascontiguousarray(np.asarray(x), dtype=np.float32)
    return [{"x": _prep_x(x[i * B_LOC:(i + 1) * B_LOC]), "w": wk, "b": bvec}
            for i in range(NCORES)]


def _unpack_y(y_s):
    # y_s [NBLK, M, SFS]; SFS = (pair16, e2, ow); m = oc*T + j
    v = y_s.reshape(NBLK, OC, T, NPAIR, BPER, OW).astype(np.float32)
    v = v.transpose(3, 4, 1, 0, 2, 5)               # pair,e,oc,blk,j,ow
    return v.reshape(B_LOC, OC, OH, OW)


def kernel(x, w3, b3, w4, b4, w6, b6):
    nc = _build()
    in_maps = prep_in_maps(x, w3, b3, w4, b4, w6, b6)
    res = run_bass_kernel_spmd(nc, in_maps, list(range(NCORES)))
    out = np.concatenate([A * _unpack_y(res.results[i]["y"])
                          for i in range(NCORES)], axis=0)
    return np.ascontiguousarray(out)
